# revision 1
# baseline (speedup 1.0000x reference)
"""Coherence-enhancing diffusion layer on 8 TRN2 NeuronCores.

Data-parallel: one 256x256 image per core, 100 diffusion iterations fully
on-chip.  All-fp32 (the 100-iteration nonlinear diffusion chaotically
amplifies any lower-precision rounding).

Structure per iteration (u stored in "R" layout [128, 512]:
tile[p, 256*b + c] = u[128*b + p, c]):
  - Sobel column convs ([-1,0,1] / [1,2,1]) as DVE free-axis shift ops.
  - Sobel row convs as banded 256x256 matmuls on PE -> gx^T, gy^T ("C"
    layout).
  - Structure-tensor products + (-rho folds via blur linearity:
    blur(q)-blur(u0) = blur(q-u0), so rho never exists).
  - 3 separable 9-tap Gaussian blurs as banded matmul pass pairs (layout
    preserved: C->R->C).
  - Pointwise nonlinearity on DVE/ACT/GPSIMD, split into halves for
    pipelining; sqrt computed as exp(0.5*ln(x)) so the ACT engine stays on
    one LUT set (a real Sqrt forces a ~1.3us table swap twice/iter).
  - delta transposed back to R via 4 PE transpose-mode matmuls; u update
    reads the transposed delta straight from PSUM.
"""
import numpy as np

N = 256
P = 128
NB = 2
ITERS = 100
RHO_INT = 4
KSIZE = 9
N_CORES = 8

_LUMA = (0.299, 0.587, 0.114)


# ---------------------------------------------------------------- host math
def _gauss1d_f32():
    half = (KSIZE - 1) / 2.0
    xs = np.linspace(-half, half, KSIZE).astype(np.float32)
    t = xs / np.float32(RHO_INT)
    pdf = np.exp(np.float32(-0.5) * t * t).astype(np.float32)
    return (pdf / pdf.sum()).astype(np.float32)


def _band_matrix(taps, pad):
    r = len(taps) // 2
    M = np.zeros((N, N), np.float64)
    for i in range(N):
        for d in range(-r, r + 1):
            j = i + d
            w = taps[d + r]
            if pad == "zero":
                if 0 <= j < N:
                    M[i, j] += w
            else:  # reflect (jnp.pad mode='reflect')
                if j < 0:
                    j = -j
                elif j >= N:
                    j = 2 * (N - 1) - j
                M[i, j] += w
    return M.astype(np.float32)


def _wt_tile(W):
    """Pack W^T into a [128, 512] block-major tile: out[p, 256k+n] = W[n, 128k+p]."""
    out = np.empty((P, NB * N), np.float32)
    for k in range(NB):
        out[:, N * k:N * (k + 1)] = W.T[P * k:P * (k + 1), :]
    return np.ascontiguousarray(out)


def _nz_ranges(W):
    """Per contraction block k: [lo, hi) range of output rows r' with any
    nonzero weight W[r', r], r in block k."""
    rngs = []
    for k in range(NB):
        nz = np.nonzero(np.any(W[:, P * k:P * (k + 1)] != 0, axis=1))[0]
        rngs.append((int(nz.min()), int(nz.max()) + 1))
    return rngs


def _pack_img(img):
    """[256,256] -> [128,512] R-layout."""
    return np.ascontiguousarray(
        img.reshape(NB, P, N).transpose(1, 0, 2).reshape(P, NB * N))


def _unpack_img(tile_arr):
    return tile_arr.reshape(P, NB, N).transpose(1, 0, 2).reshape(N, N)


_G = _band_matrix(_gauss1d_f32().astype(np.float64), "reflect")
_T = _band_matrix([1.0, 2.0, 1.0], "zero")
_D = _band_matrix([-1.0, 0.0, 1.0], "zero")


# ---------------------------------------------------------------- bass build
def _build(dt, k, iters, loop_m=1):
    import concourse.bass as bass  # noqa: F401
    import concourse.tile as tile
    from concourse import bacc, mybir

    F32 = mybir.dt.float32
    AF = mybir.ActivationFunctionType
    OP = mybir.AluOpType

    k2 = np.float32(k) * np.float32(k)
    exp_scale = float(-0.25 / k2)
    upd_scale = float(np.float32(0.5) * np.float32(dt))

    nc = bacc.Bacc("TRN2", target_bir_lowering=False, debug=False)

    # Pin all activations to the one LUT set that holds ln+exp+square+copy
    # ("natural_log_exp_and_others").  The act-table pass first-fits each
    # func over the set list, which would ping-pong between the exp-only
    # and ln-only sets (~1.3us LUT reload each time).  Emptying every other
    # set (positions preserved, so act_func_set_ids stay valid) makes the
    # fixpoint hoist a single load to kernel entry.
    import concourse.bacc as bacc_mod
    _orig_tables = bacc_mod.get_activation_tables

    def _pinned_tables(arch):
        tabs = _orig_tables(arch)
        return {
            name: (funcs if "natural_log_exp" in name else set())
            for name, funcs in tabs.items()
        }

    bacc_mod.get_activation_tables = _pinned_tables

    u0_d = nc.dram_tensor("u0", [P, NB * N], F32, kind="ExternalInput").ap()
    u0c_d = nc.dram_tensor("u0c", [P, NB * N], F32, kind="ExternalInput").ap()
    wtt_d = nc.dram_tensor("wtt", [P, NB * N], F32, kind="ExternalInput").ap()
    wtd_d = nc.dram_tensor("wtd", [P, NB * N], F32, kind="ExternalInput").ap()
    wtg_d = nc.dram_tensor("wtg", [P, NB * N], F32, kind="ExternalInput").ap()
    id_d = nc.dram_tensor("ident", [P, P], F32, kind="ExternalInput").ap()
    out_d = nc.dram_tensor("uout", [P, NB * N], F32, kind="ExternalOutput").ap()

    rng_t = _nz_ranges(_T)   # == ranges for D (same band)
    rng_g = _nz_ranges(_G)

    with tile.TileContext(nc) as tc:
        with (
            tc.tile_pool(name="consts", bufs=1) as consts,
            tc.tile_pool(name="upool", bufs=(6 if loop_m > 1 else 3)) as upool,
            tc.tile_pool(name="work", bufs=3) as work,
            tc.tile_pool(name="ps", bufs=8, space="PSUM") as psp,
        ):
            wtt = consts.tile([P, NB * N], F32)
            wtd = consts.tile([P, NB * N], F32)
            wtg = consts.tile([P, NB * N], F32)
            u0c = consts.tile([P, NB * N], F32)
            u0c2 = consts.tile([P, NB * N], F32)
            ident = consts.tile([P, P], F32)
            nc.sync.dma_start(wtt[:], wtt_d)
            nc.sync.dma_start(wtd[:], wtd_d)
            nc.sync.dma_start(wtg[:], wtg_d)
            nc.sync.dma_start(u0c[:], u0c_d)
            nc.sync.dma_start(ident[:], id_d)
            nc.vector.tensor_scalar_mul(u0c2[:], u0c[:], 2.0)

            u_t = upool.tile([P, NB * N], F32, tag="u")
            nc.sync.dma_start(u_t[:], u0_d)
            m1u0s = consts.tile([P, NB * N], F32)
            m1u0d = consts.tile([P, NB * N], F32)

            mm3 = globals().get("MM3_COMPAT", False)
            keepwarm = globals().get("KEEPWARM", True)

            def emit_section(x_tile, w_tile, rngs, psum_tile, j,
                             start_g, stop_g):
                """One output section j of psum = (W @ X)^T: 2 overlapped
                band-sliced matmuls (3 non-mixed ones under MM3_COMPAT,
                because CoreSim asserts uniform pending-zero per matmul)."""
                (lo0, hi0), (lo1, hi1) = rngs
                lhs0 = x_tile[:, N * 0 + P * j:N * 0 + P * j + P]
                lhs1 = x_tile[:, N * 1 + P * j:N * 1 + P * j + P]
                o = N * j
                nc.tensor.matmul(
                    psum_tile[:, o + lo0:o + hi0], lhs0,
                    w_tile[:, 0 + lo0:0 + hi0], start=start_g, stop=False)
                if mm3:
                    nc.tensor.matmul(
                        psum_tile[:, o + lo1:o + hi0], lhs1,
                        w_tile[:, N + lo1:N + hi0], start=False, stop=False)
                    nc.tensor.matmul(
                        psum_tile[:, o + hi0:o + N], lhs1,
                        w_tile[:, N + hi0:N + N], start=False, stop=stop_g)
                else:
                    nc.tensor.matmul(
                        psum_tile[:, o + lo1:o + N], lhs1,
                        w_tile[:, N + lo1:N + N], start=False, stop=stop_g)

            def conv_pass(x_tile, w_tile, rngs, psum_tile):
                """psum = (W @ X)^T, both sections.  One PSUM accumulation
                group spans the whole [128,512] tile (one bank/zero-region):
                start only on the first matmul, stop only on the last."""
                for j in range(NB):
                    emit_section(x_tile, w_tile, rngs, psum_tile, j,
                                 j == 0, j == NB - 1)

            def seg(ap, a, b):
                """[128,512] tile -> AP [128, 2, b-a]: free cols a..b of both
                sections."""
                return ap.rearrange("p (s c) -> p s c", s=NB)[:, :, a:b]

            def halves(ap):
                return [seg(ap, P * h, P * h + P) for h in range(NB)]

            # one-time: m1u0s = blur1(u0c), m1u0d = blur1(2*u0c); the rho
            # subtractions ride on the m1 PSUM->SBUF copies via blur
            # linearity: blur2(blur1(q) - blur1(c*u0)) = blur(q) - c*rho
            m1u_ps = psp.tile([P, NB * N], F32, tag="ps")
            conv_pass(u0c[:], wtg[:], rng_g, m1u_ps)
            nc.scalar.copy(m1u0s[:], m1u_ps[:])
            m1u2_ps = psp.tile([P, NB * N], F32, tag="ps")
            conv_pass(u0c2[:], wtg[:], rng_g, m1u2_ps)
            nc.scalar.copy(m1u0d[:], m1u2_ps[:])

            # ---- diffusion iterations ----
            # (optional outer HW loop used only for wall-clock timing runs:
            # re-runs the whole unrolled program loop_m times on-device)
            import contextlib
            outer = tc.For_i(0, loop_m, 1) if loop_m > 1 else contextlib.nullcontext()
            with outer:
                for it in range(iters):
                    u = u_t[:]
                    # Sobel column convs on DVE (free-axis shifts, zero-pad):
                    # ud = u D^T  (ud[c] = u[c+1] - u[c-1])
                    ud = work.tile([P, NB * N], F32)
                    s1 = work.tile([P, NB * N], F32)
                    ut = work.tile([P, NB * N], F32)
                    # per column-half (gates on u-stt of that half), with
                    # 3 tiny seam ops (cols 127/128) depending on both halves
                    for sx in range(NB):
                        il, ih = (1, P - 1) if sx == 0 else (P + 1, N - 1)
                        sl, sh = (0, P - 1) if sx == 0 else (P, N - 1)
                        # s1[c] = u[c] + u[c+1]
                        nc.vector.tensor_add(seg(s1[:], sl, sh),
                                             seg(u, sl, sh),
                                             seg(u, sl + 1, sh + 1))
                        # ud interior: u[c+1] - u[c-1]
                        nc.gpsimd.tensor_sub(seg(ud[:], il, ih),
                                             seg(u, il + 1, ih + 1),
                                             seg(u, il - 1, ih - 1))
                        # ut interior: s1[c-1] + s1[c]
                        nc.vector.tensor_add(seg(ut[:], il, ih),
                                             seg(s1[:], il - 1, ih - 1),
                                             seg(s1[:], il, ih))
                        if sx == 0:
                            nc.gpsimd.tensor_copy(seg(ud[:], 0, 1), seg(u, 1, 2))
                            nc.vector.tensor_add(seg(ut[:], 0, 1), seg(s1[:], 0, 1),
                                                 seg(u, 0, 1))
                        else:
                            nc.gpsimd.tensor_scalar_mul(seg(ud[:], N - 1, N),
                                                        seg(u, N - 2, N - 1), -1.0)
                            nc.vector.tensor_add(seg(ut[:], N - 1, N),
                                                 seg(s1[:], N - 2, N - 1),
                                                 seg(u, N - 1, N))
                    # seam: s1[127]; ud[127..128]; ut[127..128]
                    nc.vector.tensor_add(seg(s1[:], P - 1, P), seg(u, P - 1, P),
                                         seg(u, P, P + 1))
                    nc.gpsimd.tensor_sub(seg(ud[:], P - 1, P + 1),
                                         seg(u, P, P + 2), seg(u, P - 2, P))
                    nc.vector.tensor_add(seg(ut[:], P - 1, P + 1),
                                         seg(s1[:], P - 2, P),
                                         seg(s1[:], P - 1, P + 1))

                    # Sobel row convs on PE -> gx^T, gy^T in C layout,
                    # sections interleaved so q-phase section 0 starts early
                    gx_ps = psp.tile([P, NB * N], F32, tag="ps")
                    gy_ps = psp.tile([P, NB * N], F32, tag="ps")
                    for j in range(NB):
                        emit_section(ud[:], wtt[:], rng_t, gx_ps, j,
                                     j == 0, j == NB - 1)
                        emit_section(ut[:], wtd[:], rng_t, gy_ps, j,
                                     j == 0, j == NB - 1)

                    # structure tensor entries (C layout), with -rho folded into
                    # the blur inputs via linearity: blur(q)-blur(u0)=blur(q-u0)
                    gy_c = work.tile([P, NB * N], F32)
                    q11 = work.tile([P, NB * N], F32)
                    q22 = work.tile([P, NB * N], F32)
                    q12 = work.tile([P, NB * N], F32)
                    mq = work.tile([P, NB * N], F32)
                    pq = work.tile([P, NB * N], F32)
                    # per C-section so the B blur's k=0 matmuls can start
                    # as soon as section 0 of q12 exists
                    for sx in range(NB):
                        ss = lambda t_: t_[:, N * sx:N * sx + N]
                        nc.vector.tensor_copy(ss(gy_c), ss(gy_ps))
                        nc.scalar.activation(ss(q11), ss(gx_ps), AF.Square)
                        nc.vector.tensor_mul(ss(q12), ss(gx_ps), ss(gy_c))
                        nc.scalar.activation(ss(q22), ss(gy_c), AF.Square)
                        nc.vector.tensor_sub(ss(mq), ss(q11), ss(q22))
                        nc.gpsimd.tensor_add(ss(pq), ss(q11), ss(q22))

                    # blur pass 1+copies+pass 2, B (q12) first: the tail chain
                    # starts from B/D and can overlap the S matmuls.  The rho
                    # subtraction rides on the m1 PSUM->SBUF moves.
                    m1c_ps = psp.tile([P, NB * N], F32, tag="ps")
                    conv_pass(q12[:], wtg[:], rng_g, m1c_ps)
                    m1b_ps = psp.tile([P, NB * N], F32, tag="ps")
                    conv_pass(mq[:], wtg[:], rng_g, m1b_ps)
                    m1a_ps = psp.tile([P, NB * N], F32, tag="ps")
                    conv_pass(pq[:], wtg[:], rng_g, m1a_ps)
                    m1c_c = work.tile([P, NB * N], F32)
                    m1b_c = work.tile([P, NB * N], F32)
                    m1a_c = work.tile([P, NB * N], F32)
                    for sx in range(NB):
                        ss = lambda t_: t_[:, N * sx:N * sx + N]
                        nc.vector.tensor_sub(ss(m1c_c), ss(m1c_ps), ss(m1u0s))
                        nc.scalar.copy(ss(m1b_c), ss(m1b_ps))
                        nc.vector.tensor_sub(ss(m1a_c), ss(m1a_ps), ss(m1u0d))

                    b_ps = psp.tile([P, NB * N], F32, tag="ps")
                    conv_pass(m1c_c[:], wtg[:], rng_g, b_ps)
                    d_ps = psp.tile([P, NB * N], F32, tag="ps")
                    conv_pass(m1b_c[:], wtg[:], rng_g, d_ps)
                    s_ps = psp.tile([P, NB * N], F32, tag="ps")
                    conv_pass(m1a_c[:], wtg[:], rng_g, s_ps)

                    # pointwise tail (C layout), split into halves (rows) for
                    # pipelining; B = S12, D = S11-S22, S = S11+S22 already
                    # include the rho subtractions
                    e4 = work.tile([P, NB * N], F32)
                    d2 = work.tile([P, NB * N], F32)
                    tsum = work.tile([P, NB * N], F32)
                    lnt = work.tile([P, NB * N], F32)
                    tmp = work.tile([P, NB * N], F32)
                    a1 = work.tile([P, NB * N], F32)
                    a2 = work.tile([P, NB * N], F32)
                    sq1 = work.tile([P, NB * N], F32)
                    c1 = work.tile([P, NB * N], F32)
                    sq2 = work.tile([P, NB * N], F32)
                    c2 = work.tile([P, NB * N], F32)
                    f1 = work.tile([P, NB * N], F32)
                    f2 = work.tile([P, NB * N], F32)
                    delta_t = work.tile([P, NB * N], F32)
                    dr_ps = psp.tile([P, NB * N], F32, tag="ps")
                    if keepwarm:
                        warm_ps = psp.tile([P, NB * N], F32, tag="ps")
                    u_next = upool.tile([P, NB * N], F32, tag="u")

                    for sx in range(NB):
                        # section sx of the C layout = image column-half sx:
                        # the whole chain through transpose, u-update, and the
                        # next iteration's sobel front gates per column-half
                        hs = lambda t_: t_[:, N * sx:N * sx + N]
                        nc.scalar.activation(hs(e4), hs(b_ps), AF.Square,
                                             scale=2.0)
                        nc.scalar.activation(hs(d2), hs(d_ps), AF.Square)
                        nc.gpsimd.tensor_add(hs(tsum), hs(d2), hs(e4))
                        if keepwarm:
                            nc.tensor.transpose(
                                warm_ps[:, 0:P], tsum[:, N * sx:N * sx + P],
                                ident[:])
                        # sqrt(x) = exp(0.5*ln(x)) to stay on one ACT LUT set
                        nc.scalar.activation(hs(lnt), hs(tsum), AF.Ln)
                        nc.scalar.activation(hs(tmp), hs(lnt), AF.Exp, scale=0.5)
                        nc.vector.tensor_add(hs(a1), hs(s_ps), hs(tmp))
                        nc.vector.tensor_sub(hs(a2), hs(s_ps), hs(tmp))
                        nc.vector.tensor_mul(hs(sq1), hs(a1), hs(a1))
                        nc.scalar.activation(hs(c1), hs(sq1), AF.Exp,
                                             scale=exp_scale)
                        nc.gpsimd.tensor_mul(hs(sq2), hs(a2), hs(a2))
                        nc.scalar.activation(hs(c2), hs(sq2), AF.Exp,
                                             scale=exp_scale)
                        if keepwarm:
                            nc.tensor.transpose(
                                warm_ps[:, P:2 * P], c1[:, N * sx:N * sx + P],
                                ident[:])
                        nc.vector.tensor_mul(hs(f1), hs(c1), hs(a1))
                        nc.gpsimd.tensor_mul(hs(f2), hs(c2), hs(a2))
                        nc.vector.tensor_add(hs(delta_t), hs(f1), hs(f2))
                        # transpose delta col-half sx back to R layout; both
                        # block transposes read only section sx of delta
                        for b in range(NB):
                            nc.tensor.transpose(
                                dr_ps[:, N * b + P * sx:N * b + P * sx + P],
                                delta_t[:, N * sx + P * b:N * sx + P * b + P],
                                ident[:])
                        # u_{n+1} col-half sx = u_n + s*delta (delta straight
                        # from PSUM)
                        nc.vector.scalar_tensor_tensor(
                            halves(u_next[:])[sx], halves(dr_ps[:])[sx],
                            upd_scale, halves(u_t[:])[sx],
                            op0=OP.mult, op1=OP.add)
                    u_t = u_next

            nc.sync.dma_start(out_d, u_t[:])

    try:
        nc.compile()
    finally:
        bacc_mod.get_activation_tables = _orig_tables
    return nc


# ---------------------------------------------------------------- entry point
def _input_maps(u0, dt):
    wtt = _wt_tile(_T)
    wtd = _wt_tile(_D)
    wtg = _wt_tile(_G)
    ident = np.eye(P, dtype=np.float32)
    return [
        {"u0": _pack_img(u0[c]), "u0c": _pack_img(np.ascontiguousarray(u0[c].T)),
         "wtt": wtt, "wtd": wtd, "wtg": wtg, "ident": ident}
        for c in range(u0.shape[0])
    ]


def kernel(x, dt, k):
    from concourse.bass_utils import run_bass_kernel_spmd

    x = np.asarray(x, dtype=np.float32)
    dt_f = float(np.asarray(dt))
    k_f = float(np.asarray(k))
    B = x.shape[0]
    assert x.shape == (N_CORES, 3, N, N)

    u0 = (np.float32(_LUMA[0]) * x[:, 0] + np.float32(_LUMA[1]) * x[:, 1]
          + np.float32(_LUMA[2]) * x[:, 2]).astype(np.float32)

    nc = _build(dt_f, k_f, ITERS)

    in_maps = _input_maps(u0, dt_f)
    res = run_bass_kernel_spmd(nc, in_maps, core_ids=list(range(N_CORES)))
    u_fin = np.stack([_unpack_img(res.results[c]["uout"]) for c in range(B)])
    return np.repeat(u_fin[:, None, :, :], 3, axis=1).astype(np.float32)

